# revision 17
# baseline (speedup 1.0000x reference)
"""Trainium2 Bass kernel for nn_CrossAttention (B=2, N=2048, M=256, C=1024, H=16).

Sharding: 8 cores = 2 batches x 4 head-groups (4 heads each).
Each core computes its heads' QKV/KV projections, qk-RMSNorm, attention and a
partial output projection over its 256 channels; the host sums the 4 partials
per batch (the all-reduce) and adds proj_b.

v2: phase-3 software pipelining (logits of step s+1 issued before AV of step
s so the PE never stalls on ACT exp), fused RMSNorm (ACT Square + weight-baked
broadcast + one DVE scalar_tensor_tensor), output projection contracting 128
partitions (head pairs packed via an SBUF->SBUF DMA partition shift), and the
y-side KV work interleaved into the x QKV loop.
"""

import sys

sys.path.insert(0, "/opt/trn_rl_repo")

import numpy as np  # noqa: E402

import concourse.bass as bass  # noqa: E402
import concourse.tile as tile  # noqa: E402
from concourse import bacc, mybir  # noqa: E402
from concourse.bass_utils import run_bass_kernel_spmd  # noqa: E402

F32 = mybir.dt.float32
R32 = mybir.dt.float32r
AF = mybir.ActivationFunctionType
ADD = mybir.AluOpType.add
MUL = mybir.AluOpType.mult

H = 16
B = 2
N = 2048          # image tokens
M = 256           # text tokens
C = 1024
HD = 64           # head dim
EPS = 1e-6
S = N + M         # 2304 kv length
HPC = 4           # heads per core
NT = 512          # query tile
SCALE = HD ** -0.5


_TCNT = [0]


def T(pool, shape, tag, bufs=None, dt=F32):
    _TCNT[0] += 1
    kw = dict(tag=tag, name=f"{tag}_{_TCNT[0]}")
    if bufs is not None:
        kw["bufs"] = bufs
    return pool.tile(shape, dt, **kw)


def build_program(loop_iters=None):
    nc = bacc.Bacc("TRN2", target_bir_lowering=False, debug=False)

    xT = nc.dram_tensor("xT", [C, N], R32, kind="ExternalInput").ap()
    yT = nc.dram_tensor("yT", [C, M], R32, kind="ExternalInput").ap()
    wqkvT = nc.dram_tensor("wqkvT", [C, 2 * HPC * HD], R32, kind="ExternalInput").ap()
    bqkv = nc.dram_tensor("bqkv", [128, 4], F32, kind="ExternalInput").ap()
    wkvT = nc.dram_tensor("wkvT", [C, HPC * HD], R32, kind="ExternalInput").ap()
    wvxT = nc.dram_tensor("wvxT", [C, HPC * HD], R32, kind="ExternalInput").ap()
    wvyT = nc.dram_tensor("wvyT", [C, HPC * HD], R32, kind="ExternalInput").ap()
    bvx = nc.dram_tensor("bvx", [1, HPC * HD], R32, kind="ExternalInput").ap()
    bvy = nc.dram_tensor("bvy", [1, HPC * HD], R32, kind="ExternalInput").ap()
    ones1r = nc.dram_tensor("ones1r", [1, 128], R32, kind="ExternalInput").ap()
    bkv = nc.dram_tensor("bkv", [128, 2], F32, kind="ExternalInput").ap()
    wproj128 = nc.dram_tensor("wproj128", [128, 2, C], R32, kind="ExternalInput").ap()
    onesb = nc.dram_tensor("onesb", [128, 2], R32, kind="ExternalInput").ap()
    ones2q = nc.dram_tensor("ones2q", [2, 128], R32, kind="ExternalInput").ap()
    ones2k = nc.dram_tensor("ones2k", [2, 128], R32, kind="ExternalInput").ap()
    sel64 = nc.dram_tensor("sel64", [65, 64], R32, kind="ExternalInput").ap()
    vones = nc.dram_tensor("vones", [128, 18, 1], R32, kind="ExternalInput").ap()
    outT = nc.dram_tensor("outT", [C, N], F32, kind="ExternalOutput").ap()

    with tile.TileContext(nc) as tc:
        with (
            tc.tile_pool(name="const", bufs=1) as const,
            tc.tile_pool(name="sing", bufs=1) as sing,
            tc.tile_pool(name="xin", bufs=2) as xin,
        ):
            # DMA issue order is queue order: load what the first QKV matmul
            # needs first (wqkv, biases, norm consts, x tile 0), then the
            # v-projection weights, then the y-side tensors (used at nt==1),
            # then attention/proj consts.
            wqkv_sb = T(const, [128, 8, 2 * HPC * HD], "wqkv", dt=R32)
            wqkv_r = wqkvT.rearrange("(o p) f -> p o f", p=128)
            for cc in range(8):
                nc.sync.dma_start(wqkv_sb[:, cc], wqkv_r[:, cc])
            bqkv_sb = T(const, [128, 4], "bqkv")
            nc.sync.dma_start(bqkv_sb, bqkv)
            onesb_sb = T(const, [128, 2], "onesb", dt=R32)
            nc.sync.dma_start(onesb_sb, onesb)
            ones2q_sb = T(const, [2, 128], "ones2q", dt=R32)
            nc.sync.dma_start(ones2q_sb, ones2q)
            ones2k_sb = T(const, [2, 128], "ones2k", dt=R32)
            nc.sync.dma_start(ones2k_sb, ones2k)

            xc0 = T(xin, [128, 8, NT], "xc", dt=R32)
            xT_r = xT.rearrange("(o p) f -> p o f", p=128)
            for cc in range(8):
                nc.sync.dma_start(xc0[:, cc], xT_r[:, cc, 0:NT])

            wvx_sb = T(const, [128, 8, HPC * HD], "wvx", dt=R32)
            nc.sync.dma_start(wvx_sb, wvxT.rearrange("(o p) f -> p o f", p=128))
            bvx_sb = T(const, [1, HPC * HD], "bvx", dt=R32)
            nc.sync.dma_start(bvx_sb, bvx)
            ones1_sb = T(const, [1, 128], "ones1r", dt=R32)
            nc.sync.dma_start(ones1_sb, ones1r)

            # persistent activations: channel-on-partition layouts
            qT = T(sing, [128, 2, N], "qT", dt=R32)       # [2 heads x 64d, hp, n]
            kT = T(sing, [128, 2, S], "kT", dt=R32)
            vS = T(sing, [128, 18, HPC * 65], "vS", dt=R32)  # [s%128, s//128, h*65+(d|one)]
            for h in range(HPC):
                nc.sync.dma_start(vS[:, :, 65 * h + 64 : 65 * h + 65], vones)

            yT_sb = T(const, [128, 8, M], "yT", dt=R32)
            nc.sync.dma_start(yT_sb, yT.rearrange("(o p) f -> p o f", p=128))
            wkv_sb = T(const, [128, 8, HPC * HD], "wkv", dt=R32)
            nc.sync.dma_start(wkv_sb, wkvT.rearrange("(o p) f -> p o f", p=128))
            bkv_sb = T(const, [128, 2], "bkv")
            nc.sync.dma_start(bkv_sb, bkv)
            wvy_sb = T(const, [128, 8, HPC * HD], "wvy", dt=R32)
            nc.sync.dma_start(wvy_sb, wvyT.rearrange("(o p) f -> p o f", p=128))
            bvy_sb = T(const, [1, HPC * HD], "bvy", dt=R32)
            nc.sync.dma_start(bvy_sb, bvy)
            sel_sb = T(const, [65, 64], "sel64", dt=R32)
            nc.sync.dma_start(sel_sb, sel64)
            wproj_sb = T(const, [128, 2, C], "wproj", dt=R32)
            nc.sync.dma_start(wproj_sb, wproj128)
            eps_sb = T(const, [128, 1], "epsc")
            nc.vector.memset(eps_sb, float(EPS))
            zero_sb = T(const, [128, 1], "zeroc")
            nc.vector.memset(zero_sb, 0.0)

            def norm_chunk(pool_ps, pool_wk, psum, bias_ap, rbcw_sb, dest):
                """dest = (psum + bias) * rsqrt(mean_d((psum+bias)^2)+eps) * w

                sq via ACT Square (bias folded in); rms broadcast via matmul
                with the norm weight baked into the [2,128] stationary; final
                scaling as one fused DVE op.
                """
                nsz = psum.shape[-1]
                sq = T(pool_wk, [128, NT], "w", bufs=4, dt=R32)[:, :nsz]
                nc.scalar.activation(sq, psum, AF.Square, bias=bias_ap, scale=1.0)
                ssp = T(pool_ps, [2, NT], "paux", bufs=3)[:, :nsz]
                nc.tensor.matmul(ssp, onesb_sb, sq, start=True, stop=True)
                lnv = T(pool_wk, [2, NT], "w2", bufs=4)[:, :nsz]
                nc.scalar.activation(
                    lnv, ssp, AF.Ln, bias=eps_sb[0:2], scale=1.0 / HD
                )
                rmsv = T(pool_wk, [2, NT], "w2", bufs=4, dt=R32)[:, :nsz]
                nc.scalar.activation(rmsv, lnv, AF.Exp, bias=zero_sb[0:2], scale=-0.5)
                rbc = T(pool_ps, [128, NT], "paux", bufs=3)[:, :nsz]
                nc.tensor.matmul(rbc, rbcw_sb, rmsv, start=True, stop=True)
                # DVE reads at most one non-scalar input from PSUM: stage the
                # broadcast through SBUF before the fused (psum+bias)*rbc op.
                rbs = T(pool_wk, [128, NT], "w", bufs=4, dt=R32)[:, :nsz]
                nc.vector.tensor_copy(rbs, rbc)
                nc.vector.scalar_tensor_tensor(dest, psum, bias_ap, rbs, ADD, MUL)

            def v_proj(pool_ps, src_sb, t, w_sb, b_sb, j):
                """vS[:, j] = (src.T @ wv + bv) directly in [s, d] layout."""
                pv = T(pool_ps, [128, HPC * HD], "pmain", bufs=4)
                for cc in range(8):
                    nc.tensor.matmul(
                        pv,
                        src_sb[:, cc, t * 128 : (t + 1) * 128],
                        w_sb[:, cc, :],
                        start=(cc == 0),
                        stop=False,
                    )
                nc.tensor.matmul(pv, ones1_sb, b_sb, start=False, stop=True)
                dst = vS[:, j, :].rearrange("p (a b) -> p a b", b=65)[:, :, 0:64]
                nc.vector.tensor_copy(
                    out=dst, in_=pv.rearrange("p (a b) -> p a b", b=64)
                )

            import contextlib
            with contextlib.ExitStack() as _les:
                if loop_iters is not None:
                    _les.enter_context(tc.For_i(0, loop_iters, 1))
                # ---- phase 1+2: QKV projection of x, with y's KV interleaved
                with (
                    tc.tile_pool(name="pp12", bufs=3, space="PSUM") as pp12,
                    tc.tile_pool(name="wk", bufs=12) as wk,
                ):
                    for nt in range(N // NT):
                        nsl = slice(nt * NT, (nt + 1) * NT)
                        if nt == 0:
                            xc = xc0
                        else:
                            xc = T(xin, [128, 8, NT], "xc", dt=R32)
                            for cc in range(8):
                                nc.sync.dma_start(xc[:, cc], xT_r[:, cc, nsl])
                        for mc in range(4):  # [q01,q23,k01,k23]
                            ps = T(pp12, [128, NT], "pmain", bufs=4)
                            for cc in range(8):
                                nc.tensor.matmul(
                                    ps,
                                    wqkv_sb[:, cc, mc * 128 : (mc + 1) * 128],
                                    xc[:, cc, :],
                                    start=(cc == 0),
                                    stop=(cc == 7),
                                )
                            bias_ap = bqkv_sb[:, mc : mc + 1]
                            if mc < 2:
                                norm_chunk(pp12, wk, ps, bias_ap,
                                           ones2q_sb, qT[:, mc, nsl])
                            else:
                                norm_chunk(pp12, wk, ps, bias_ap,
                                           ones2k_sb, kT[:, mc - 2, nsl])
                        for t in range(4):
                            v_proj(pp12, xc, t, wvx_sb, bvx_sb, nt * 4 + t)
                        if nt == 1:
                            # y-side KV (text tokens -> kv rows 2048..2303)
                            for mc in range(2):  # [k01, k23]
                                ps = T(pp12, [128, NT], "pmain", bufs=4)[:, :M]
                                for cc in range(8):
                                    nc.tensor.matmul(
                                        ps,
                                        wkv_sb[:, cc, mc * 128 : (mc + 1) * 128],
                                        yT_sb[:, cc, :],
                                        start=(cc == 0),
                                        stop=(cc == 7),
                                    )
                                norm_chunk(
                                    pp12, wk, ps, bkv_sb[:, mc : mc + 1],
                                    ones2k_sb, kT[:, mc, N : N + M],
                                )
                            for t in range(2):
                                v_proj(pp12, yT_sb, t, wvy_sb, bvy_sb, 16 + t)

                # ---- phase 3+4: attention + output projection, per query tile
                # PSUM layout in this phase (8 banks): bigA [128,3*NT] x1 (3)
                # + bigB [128,2*NT] x1 (2) + av accumulators [128,NT] x2 (2)
                # + po/dbc [128,NT] x1 (1). Logits steps alternate bigA/bigB
                # (s-tile widths 3,2,3,2,3,2,3) so the po ring never blocks
                # the AV accumulators of the next tile.
                WIDTHS = [3, 2, 3, 2, 3, 2, 3]
                with (
                    tc.tile_pool(name="pa", bufs=1, space="PSUM") as pa,
                    tc.tile_pool(name="atp", bufs=3) as atp,
                    tc.tile_pool(name="asp", bufs=3) as asp,
                    tc.tile_pool(name="outp", bufs=2) as outp,
                    tc.tile_pool(name="osp", bufs=2) as osp,
                ):
                    def emit_av(pend):
                        at, j0, w, idx, av_list, hp = pend
                        h = 2 * hp + idx
                        for u in range(w):
                            nc.tensor.matmul(
                                av_list[idx],
                                vS[:, j0 + u, 65 * h : 65 * h + 65],
                                at[:, u * NT : (u + 1) * NT],
                                start=(j0 + u == 0),
                                stop=(j0 + u == 17),
                            )

                    def make_tail(hp, av_list, ot):
                        def tail():
                            for idx in range(2):
                                avs = T(asp, [65, NT], "avs", dt=R32)
                                nc.vector.tensor_copy(avs, av_list[idx])
                                dbc = T(pa, [64, NT], "po", bufs=1)
                                nc.tensor.matmul(
                                    dbc, sel_sb, avs, start=True, stop=True
                                )
                                rbc = T(asp, [64, NT], "rbc")
                                nc.vector.reciprocal(rbc, dbc)
                                if idx == 0:
                                    nc.vector.tensor_mul(
                                        ot[0:64, hp, :], avs[0:64, :], rbc
                                    )
                                else:
                                    tmp = T(asp, [64, NT], "otmp", dt=R32)
                                    nc.vector.tensor_mul(tmp, avs[0:64, :], rbc)
                                    nc.sync.dma_start(ot[64:128, hp, :], tmp)
                        return tail

                    def make_proj_mini(nt, ot, oc):
                        nsl = slice(nt * NT, (nt + 1) * NT)
                        def mini():
                            po = T(pa, [128, NT], "po", bufs=1)
                            for g in range(2):
                                nc.tensor.matmul(
                                    po,
                                    wproj_sb[:, g, oc * 128 : (oc + 1) * 128],
                                    ot[:, g, :],
                                    start=(g == 0), stop=(g == 1),
                                )
                            ob = T(osp, [128, NT], "ob")
                            nc.vector.tensor_copy(ob, po)
                            nc.sync.dma_start(
                                outT.rearrange("(o p) f -> p o f", p=128)[:, oc, nsl],
                                ob,
                            )
                        return mini

                    pending = []       # exp outputs awaiting their AV matmuls
                    deferred_tail = [None]  # division tail of the previous block
                    minis = []         # per-oc projection chunks to drip-feed
                    ot = None
                    for nt in range(N // NT):
                        nsl = slice(nt * NT, (nt + 1) * NT)
                        for hp in range(2):
                            if hp == 0:
                                ot = T(outp, [128, 2, NT], "ot", bufs=3, dt=R32)
                            av_list = [
                                T(pa, [128, NT], "avac", bufs=2)[:65],
                                T(pa, [128, NT], "avac", bufs=2)[:65],
                            ]
                            j0s = [0, 0]
                            for si, w in enumerate(WIDTHS):
                                for idx in range(2):
                                    j0 = j0s[idx]
                                    prt = slice(64 * idx, 64 * idx + 64)
                                    tp = (64 * idx, 0)
                                    if w == 3:
                                        pl = T(pa, [128, 3 * NT], "bigA", bufs=1)
                                    else:
                                        pl = T(pa, [128, 2 * NT], "bigB", bufs=1)
                                    rhsQ = qT[prt, hp, nsl]
                                    for u in range(w):
                                        nc.tensor.matmul(
                                            pl[:, u * NT : (u + 1) * NT],
                                            kT[prt, hp,
                                               (j0 + u) * 128 : (j0 + u + 1) * 128],
                                            rhsQ, start=True, stop=True,
                                            tile_position=tp,
                                        )
                                    at = T(atp, [128, 3 * NT], "at", bufs=4, dt=R32)
                                    nc.scalar.activation(
                                        at[:, : w * NT], pl, AF.Exp,
                                        bias=zero_sb[:], scale=SCALE,
                                    )
                                    pending.append((at, j0, w, idx, av_list, hp))
                                    j0s[idx] += w
                                    if len(pending) > 2:
                                        emit_av(pending.pop(0))
                                if si == 0 and deferred_tail[0] is not None:
                                    deferred_tail[0]()
                                    deferred_tail[0] = None
                                elif si >= 1 and minis:
                                    minis.pop(0)()
                            deferred_tail[0] = make_tail(hp, av_list, ot)
                            if hp == 1:
                                minis.extend(
                                    make_proj_mini(nt, ot, oc) for oc in range(8)
                                )
                    while pending:
                        emit_av(pending.pop(0))
                    if deferred_tail[0] is not None:
                        deferred_tail[0]()
                    while minis:
                        minis.pop(0)()
    _orig = bacc.get_activation_tables

    def _tables(arch):
        t = _orig(arch)
        return {
            name: (set() if name in ("exp_and_others", "natural_log",
                                     "exp_and_friends") else fns)
            for name, fns in t.items()
        }

    bacc.get_activation_tables = _tables
    try:
        nc.compile()
    finally:
        bacc.get_activation_tables = _orig
    return nc


_PROGRAM = None


def _get_program():
    global _PROGRAM
    if _PROGRAM is None:
        _PROGRAM = build_program()
    return _PROGRAM


def _make_in_maps(x, y, qkv_w, qkv_b, kv_w, kv_b, qn_w, kn_w, proj_w, proj_b):
    f = np.float32
    onesb = np.zeros((128, 2), f)
    onesb[0:64, 0] = 1.0
    onesb[64:128, 1] = 1.0
    ones2q = np.zeros((2, 128), f)
    ones2q[0, 0:64] = qn_w
    ones2q[1, 64:128] = qn_w
    ones2k = np.zeros((2, 128), f)
    ones2k[0, 0:64] = kn_w
    ones2k[1, 64:128] = kn_w
    sel64 = np.zeros((65, 64), f)
    sel64[64, :] = 1.0

    in_maps = []
    for core in range(8):
        b, g = divmod(core, 4)
        qs = slice(g * 256, (g + 1) * 256)
        wqkv = np.concatenate([qkv_w[qs], qkv_w[1024:2048][qs]], axis=0)
        bq = np.concatenate([qkv_b[qs], qkv_b[1024:2048][qs]])
        wkv = kv_w[qs]
        bk = kv_b[qs]
        wvx = qkv_w[2048:3072][qs]
        bvxv = qkv_b[2048:3072][qs]
        wvy = kv_w[1024:2048][qs]
        bvyv = kv_b[1024:2048][qs]
        wp = np.ascontiguousarray(
            proj_w[:, qs].T.reshape(2, 128, C).transpose(1, 0, 2), f
        )
        in_maps.append(
            {
                "xT": np.ascontiguousarray(x[b].T, f),
                "yT": np.ascontiguousarray(y[b].T, f),
                "wqkvT": np.ascontiguousarray(wqkv.T, f),
                "bqkv": np.ascontiguousarray(bq.reshape(4, 128).T, f),
                "wkvT": np.ascontiguousarray(wkv.T, f),
                "bkv": np.ascontiguousarray(bk.reshape(2, 128).T, f),
                "wvxT": np.ascontiguousarray(wvx.T, f),
                "bvx": np.ascontiguousarray(bvxv.reshape(1, 256), f),
                "wvyT": np.ascontiguousarray(wvy.T, f),
                "bvy": np.ascontiguousarray(bvyv.reshape(1, 256), f),
                "ones1r": np.ones((1, 128), f),
                "wproj128": wp,
                "onesb": onesb,
                "ones2q": ones2q,
                "ones2k": ones2k,
                "sel64": sel64,
                "vones": np.ones((128, 18, 1), f),
            }
        )
    return in_maps


def run_cores(inputs, trace=False, **kwargs):
    nc = _get_program()
    in_maps = _make_in_maps(**{k: np.asarray(v, np.float32) for k, v in inputs.items()})
    return run_bass_kernel_spmd(
        nc, in_maps, core_ids=list(range(8)), trace=trace, **kwargs
    )


def kernel(**inputs):
    proj_b = np.asarray(inputs["proj_b"], np.float32)
    res = run_cores(inputs).results
    out = np.zeros((B, N, C), np.float32)
    for core in range(8):
        b = core // 4
        out[b] += res[core]["outT"].T
    out += proj_b[None, None, :]
    return out
